# revision 3
# baseline (speedup 1.0000x reference)
"""Trainium2 Bass kernel for nn_BaselineAttn (LoRA QKV + ALiBi causal attention).

Sharding: 8 cores SPMD, no collectives. Core c = (b, g): batch b = c // 4,
head group g = c % 4 handling heads [g, 4+g, 8+g, 12+g].

Host prep: LoRA folded into weights (W' = W + 2 A@B); x and weights
pre-transposed/sliced per core; partial outputs (fp16) summed on host.

v2 design (fused single-pass pipeline, fp16 operands, fp32 PSUM):
  - One pool set for the whole program (no phase barriers): QKV projection,
    attention and output projection overlap; ScalarE exp starts ~10us in.
  - Merged q-chunks of 1024: one exp ACTIVATE per (slot, key-tile) covers
    [128, <=1024] spanning 2 PSUM banks -> half the ACTIVATE fixed overhead.
  - S^T pairs (slot even/odd) interleave at base partitions 0/64 so the
    K=64 matmuls pack into row groups (0,0)/(64,0) and run concurrently;
    stE/stO ping-pong keeps ScalarE exp streaming without st double-buffer.
  - Causal mask multiply only touches the [128,128] triangle block.
  - PV even-slot streams in the kt loop; odd-slot P tiles are retained in
    SBUF and PV-odd batches after even's normalize frees the ot banks.
  - Denominator via ones-column in vext (row 64 of O^T); reciprocal on DVE,
    partition-broadcast via small DRAM bounce, normalize on DVE.
  - PSUM->SBUF copies on GpSimd; proj results converted to fp16 on DVE.
  - ALiBi decay (reference adds slope*(i-j), so old keys dominate): keys with
    slope_h*k > ~45 dropped; per-slot key-tile caps SNKT = [2, 6, 16, 16].
"""

import math

import numpy as np

E = 1024
H = 16
DH = 64
T = 2048
BATCH = 2
LORA_S = 2.0
SNKT = [2, 6, 16, 16]   # per-slot key-tile caps (uniform across cores)
MQ = 2                  # merged q-chunks of 1024

_NC_CACHE = {}


def _slopes():
    start = 2 ** (-2 ** (-(math.log2(H) - 3)))
    return np.array([start * start**i for i in range(H)], dtype=np.float64)


def _smin(tt):
    """Lowest slot that still needs key-tile tt."""
    for s in range(4):
        if tt < SNKT[s]:
            return s
    return 4


def _build_nc():
    if "nc" in _NC_CACHE:
        return _NC_CACHE["nc"]

    from concourse.bacc import Bacc
    import concourse.tile as tile
    from concourse import mybir

    f16 = mybir.dt.float16
    f32 = mybir.dt.float32
    EXP = mybir.ActivationFunctionType.Exp

    nc = Bacc()

    xT_d = nc.dram_tensor("xT", [E, T], f16, kind="ExternalInput")
    wqkv_d = nc.dram_tensor("wqkvT", [E, 768], f16, kind="ExternalInput")
    wp_d = nc.dram_tensor("wpT", [256, E], f16, kind="ExternalInput")
    bias_d = nc.dram_tensor("expbias", [128, 64], f32, kind="ExternalInput")
    mask_d = nc.dram_tensor("masktri", [128, 128], f16, kind="ExternalInput")
    out_d = nc.dram_tensor("outp", [T, E], f16, kind="ExternalOutput")
    rb_d = nc.dram_tensor("rbounce", [8, 1024], f32, kind="Internal")

    with tile.TileContext(nc) as tc:
        with (
            tc.tile_pool(name="persist", bufs=1) as pp,
            tc.tile_pool(name="pe_pool", bufs=3) as pep,
            tc.tile_pool(name="po_pool", bufs=16) as pop,
            tc.tile_pool(name="rp", bufs=2) as rp,
            tc.tile_pool(name="osp", bufs=4) as osp,
            tc.tile_pool(name="stE", bufs=1, space="PSUM") as stEp,
            tc.tile_pool(name="stO", bufs=1, space="PSUM") as stOp,
            tc.tile_pool(name="otp", bufs=1, space="PSUM") as otp,
            tc.tile_pool(name="auxp", bufs=2, space="PSUM") as auxp,
        ):
            # ---- persistent SBUF ----
            xsb = pp.tile([128, 8, T], f16, name="xsb")
            wsb = pp.tile([128, 8, 768], f16, name="wsb")
            wpsb = pp.tile([128, 2, E], f16, name="wpsb")
            bias_sb = pp.tile([128, 64], f32, name="bias")
            mask_sb = pp.tile([128, 128], f16, name="masktri")
            qT = [[pp.tile([128, 512], f16, name=f"qT{p}_{c}") for c in range(4)]
                  for p in range(2)]
            kT = [[pp.tile([128, 512], f16, name=f"kT{p}_{c}")
                   if (p == 1 or c < 2) else None for c in range(4)]
                  for p in range(2)]
            vext = [pp.tile([128, 4, 65], f16, name=f"vext{t}") for t in range(16)]
            on_t = [[pp.tile([128, 1024], f16, name=f"on{p}_{m}") for m in range(MQ)]
                    for p in range(2)]

            # dummy exp to pull the ACT table load off the critical path
            scr = pp.tile([1, 2], f32, name="scr")
            nc.vector.memset(scr, 0.0)
            nc.scalar.activation(out=scr[0:1, 1:2], in_=scr[0:1, 0:1], func=EXP)

            for t in range(16):
                nc.gpsimd.memset(vext[t], 1.0)  # ones col preset; v overwrites rest

            # ---- input DMAs, ordered for earliest compute start ----
            nc.sync.dma_start(out=bias_sb, in_=bias_d[:, :])
            nc.sync.dma_start(out=mask_sb, in_=mask_d[:, :])
            x3 = xT_d.rearrange("(k p) t -> p k t", p=128)
            w3 = wqkv_d.rearrange("(k p) c -> p k c", p=128)
            wp3 = wp_d.rearrange("(k p) c -> p k c", p=128)
            nc.sync.dma_start(out=wsb[:, :, 0:256], in_=w3[:, :, 0:256])
            nc.sync.dma_start(out=wsb[:, :, 256:512], in_=w3[:, :, 256:512])
            nc.sync.dma_start(out=wsb[:, :, 512:768], in_=w3[:, :, 512:768])
            nc.sync.dma_start(out=xsb[:, :, 0:1024], in_=x3[:, :, 0:1024])
            nc.sync.dma_start(out=wpsb, in_=wp3)
            nc.sync.dma_start(out=xsb[:, :, 1024:2048], in_=x3[:, :, 1024:2048])

            def phase1(mq):
                """QKV projections for q-chunks 2mq, 2mq+1 (v key-tiles 8mq..8mq+7)."""
                with nc.named_scope(f"qkv_mq{mq}"):
                    for ncu in (2 * mq, 2 * mq + 1):
                        for wofs, dst in ((0, qT), (256, kT)):
                            for mt in range(2):
                                if dst[mt][ncu] is None:
                                    continue
                                nw = 256 if (wofs == 256 and mt == 0 and ncu == 1) else 512
                                acc = auxp.tile([128, 512], f32, tag="aux",
                                                name=f"qk{wofs}_{mt}_{ncu}")
                                for kt in range(8):
                                    nc.tensor.matmul(
                                        acc[:, 0:nw],
                                        wsb[:, kt, wofs + mt * 128:wofs + (mt + 1) * 128],
                                        xsb[:, kt, ncu * 512:ncu * 512 + nw],
                                        start=(kt == 0), stop=(kt == 7),
                                    )
                                nc.vector.tensor_copy(out=dst[mt][ncu][:, 0:nw],
                                                      in_=acc[:, 0:nw])
                        for tt in range(4 * ncu, 4 * ncu + 4):
                            s0 = _smin(tt)
                            nw = (4 - s0) * 64
                            acc = auxp.tile([128, 512], f32, tag="aux", name=f"vacc{tt}")
                            for kt in range(8):
                                nc.tensor.matmul(
                                    acc[:, 0:nw],
                                    xsb[:, kt, tt * 128:(tt + 1) * 128],
                                    wsb[:, kt, 512 + s0 * 64:768],
                                    start=(kt == 0), stop=(kt == 7),
                                )
                            nc.vector.tensor_copy(
                                out=vext[tt][:, s0:4, 0:64],
                                in_=acc[:, 0:nw].rearrange("p (s d) -> p s d", d=64))

            def norm(ot, mq, pair, s, r0):
                """on[r0:r0+64] = ot[0:64] / ot[64] (denominator row)."""
                row = 4 * mq + s
                sum_sb = rp.tile([1, 1024], f32, tag="sum", name=f"sum_{mq}_{s}")
                nc.vector.tensor_copy(out=sum_sb, in_=ot[64:65, 0:1024])
                rec = rp.tile([1, 1024], f32, tag="rec", name=f"rec_{mq}_{s}")
                nc.vector.reciprocal_approx_fast(out=rec, in_=sum_sb)
                nc.sync.dma_start(out=rb_d[row:row + 1, :], in_=rec)
                bcs = rp.tile([128, 1024], f32, tag="bcs", name=f"bcs_{mq}_{s}")
                nc.sync.dma_start(
                    out=bcs[r0:r0 + 64, :],
                    in_=rb_d[row:row + 1, :].to_broadcast([64, 1024]))
                nc.vector.tensor_mul(
                    out=on_t[pair][mq][r0:r0 + 64, :],
                    in0=ot[0:64, :], in1=bcs[r0:r0 + 64, :])

            def attention(mq):
                for pair in (1, 0):  # big-exp pair first
                    se, so = 2 * pair, 2 * pair + 1
                    pt = pair
                    nkt_e = min(SNKT[se], 8 * mq + 8)
                    nkt_o = min(SNKT[so], 8 * mq + 8)
                    ot_e = otp.tile([128, 1024], f32, tag="ot", name=f"ote_{mq}_{pair}")
                    p_o_tiles = []
                    with nc.named_scope(f"attn_mq{mq}_p{pair}"):
                        for kt in range(nkt_o):
                            j0 = max(0, (kt - 8 * mq) * 128)
                            kc, ko = kt // 4, (kt % 4) * 128
                            st_e = (stEp.tile([128, 1024], f32, tag="st",
                                              name=f"ste_{mq}_{pair}_{kt}")
                                    if kt < nkt_e else None)
                            st_o = stOp.tile([128, 1024], f32, tag="st",
                                             name=f"sto_{mq}_{pair}_{kt}")
                            for h in (0, 1):
                                lo, hi = max(j0, 512 * h), 512 * (h + 1)
                                if lo >= hi:
                                    continue
                                for st, r0 in ((st_e, 0), (st_o, 64)):
                                    if st is None:
                                        continue
                                    nc.tensor.matmul(
                                        st[:, lo:hi],
                                        kT[pt][kc][r0:r0 + 64, ko:ko + 128],
                                        qT[pt][2 * mq + h][r0:r0 + 64, lo - 512 * h:512],
                                        start=True, stop=True,
                                    )
                            for st, s, r0, is_e in ((st_e, se, 0, True),
                                                    (st_o, so, 64, False)):
                                if st is None:
                                    continue
                                pool, tg = (pep, "pe") if is_e else (pop, "po")
                                p_t = pool.tile([128, 1024], f16, tag=tg,
                                                name=f"p{tg}_{mq}_{pair}_{kt}")
                                nc.scalar.activation(
                                    out=p_t[:, j0:1024], in_=st[:, j0:1024],
                                    func=EXP,
                                    bias=bias_sb[:, s * 16 + kt:s * 16 + kt + 1],
                                    scale=0.125,
                                )
                                if kt >= 8 * mq:
                                    nc.gpsimd.tensor_mul(
                                        out=p_t[:, j0:j0 + 128],
                                        in0=p_t[:, j0:j0 + 128], in1=mask_sb)
                                if is_e:
                                    for h in (0, 1):
                                        lo, hi = max(j0, 512 * h), 512 * (h + 1)
                                        if lo >= hi:
                                            continue
                                        last = kt == ((min(nkt_e, 8 * mq + 4) - 1)
                                                      if h == 0 else nkt_e - 1)
                                        nc.tensor.matmul(
                                            ot_e[0:65, lo:hi],
                                            vext[kt][:, se, :], p_t[:, lo:hi],
                                            start=(kt == 0), stop=last,
                                        )
                                else:
                                    p_o_tiles.append((kt, j0, p_t))
                        norm(ot_e, mq, pair, se, 0)
                        ot_o = otp.tile([128, 1024], f32, tag="ot",
                                        name=f"oto_{mq}_{pair}")
                        for kt, j0, p_t in p_o_tiles:
                            for h in (0, 1):
                                lo, hi = max(j0, 512 * h), 512 * (h + 1)
                                if lo >= hi:
                                    continue
                                last = kt == ((min(nkt_o, 8 * mq + 4) - 1)
                                              if h == 0 else nkt_o - 1)
                                nc.tensor.matmul(
                                    ot_o[0:65, lo:hi],
                                    vext[kt][:, so, :], p_t[:, lo:hi],
                                    start=(kt == 0), stop=last,
                                )
                        norm(ot_o, mq, pair, so, 64)

            def proj(mq):
                with nc.named_scope(f"proj_mq{mq}"):
                    for tloc in range(8):
                        tt = 8 * mq + tloc
                        for ech in range(2):
                            pacc = auxp.tile([128, 512], f32, tag="aux",
                                             name=f"pacc_{tt}_{ech}")
                            for pt in range(2):
                                nc.tensor.matmul(
                                    pacc,
                                    on_t[pt][mq][:, tloc * 128:(tloc + 1) * 128],
                                    wpsb[:, pt, ech * 512:(ech + 1) * 512],
                                    start=(pt == 0), stop=(pt == 1),
                                )
                            osb = osp.tile([128, 512], f16, tag="osb",
                                           name=f"osb_{tt}_{ech}")
                            nc.vector.tensor_copy(out=osb, in_=pacc)
                            nc.sync.dma_start(
                                out=out_d[tt * 128:(tt + 1) * 128,
                                          ech * 512:(ech + 1) * 512],
                                in_=osb)

            for mq in range(MQ):
                phase1(mq)
                attention(mq)
                proj(mq)

    nc.finalize()
    _NC_CACHE["nc"] = nc
    return nc


def _prep_core_inputs(x, Wq, Aq, Bq, Wk, Ak, Bk, Wv, Av, Bv, Wp):
    """Host-side prep: LoRA fold, transposes, per-core slices."""
    slopes = _slopes()
    wq_m = Wq.astype(np.float64) + LORA_S * (Aq.astype(np.float64) @ Bq.astype(np.float64))
    wk_m = Wk.astype(np.float64) + LORA_S * (Ak.astype(np.float64) @ Bk.astype(np.float64))
    wv_m = Wv.astype(np.float64) + LORA_S * (Av.astype(np.float64) @ Bv.astype(np.float64))

    # triangle mask[p, j] = 1 if p <= j (valid, key on/before query) else 0
    p_i = np.arange(128)[:, None]
    j_i = np.arange(128)[None, :]
    masktri = np.ascontiguousarray((p_i <= j_i).astype(np.float16))

    in_maps = []
    for c in range(8):
        b, g = divmod(c, 4)
        heads = [g, 4 + g, 8 + g, 12 + g]
        rows = np.concatenate([np.arange(h * DH, (h + 1) * DH) for h in heads])
        xT = np.ascontiguousarray(x[b].T.astype(np.float16))
        wqkvT = np.ascontiguousarray(np.concatenate(
            [wq_m[rows, :].T, wk_m[rows, :].T, wv_m[rows, :].T],
            axis=1).astype(np.float16))
        wpT = np.ascontiguousarray(Wp[:, rows].T.astype(np.float16))
        bias = np.zeros((128, 64), dtype=np.float32)
        for s, h in enumerate(heads):
            for kt in range(16):
                bias[:, s * 16 + kt] = -slopes[h] * (kt * 128 + np.arange(128))
        in_maps.append({
            "xT": xT, "wqkvT": wqkvT, "wpT": wpT,
            "expbias": bias, "masktri": masktri,
        })
    return in_maps


def _run(in_maps, trace=False, **kw):
    from concourse.bass_utils import run_bass_kernel_spmd
    nc = _build_nc()
    return run_bass_kernel_spmd(nc, in_maps, core_ids=list(range(8)), trace=trace, **kw)


def kernel(x, Wq, Aq, Bq, Wk, Ak, Bk, Wv, Av, Bv, Wp):
    in_maps = _prep_core_inputs(x, Wq, Aq, Bq, Wk, Ak, Bk, Wv, Av, Bv, Wp)
    res = _run(in_maps)
    out = np.zeros((BATCH, T, E), dtype=np.float32)
    for c in range(8):
        out[c // 4] += res.results[c]["outp"].astype(np.float32)
    return out


# revision 8
# speedup vs baseline: 1.1249x; 1.1249x over previous
"""Trainium2 Bass kernel for nn_BaselineAttn (LoRA QKV + ALiBi causal attention).

Sharding: 8 cores SPMD, no collectives. Core c = (b, g): batch b = c // 4,
head group g = c % 4 handling heads [g, 4+g, 8+g, 12+g].

Host prep: LoRA folded into weights (W' = W + 2 A@B); x and weights
pre-transposed/sliced per core; partial outputs (fp16) summed on host.

v2 design (fused single-pass pipeline, fp16 operands, fp32 PSUM):
  - One pool set for the whole program (no phase barriers): QKV projection,
    attention and output projection overlap; ScalarE exp starts ~10us in.
  - Merged q-chunks of 1024: one exp ACTIVATE per (slot, key-tile) covers
    [128, <=1024] spanning 2 PSUM banks -> half the ACTIVATE fixed overhead.
  - S^T pairs (slot even/odd) interleave at base partitions 0/64 so the
    K=64 matmuls pack into row groups (0,0)/(64,0) and run concurrently;
    stE/stO ping-pong keeps ScalarE exp streaming without st double-buffer.
  - Causal mask multiply only touches the [128,128] triangle block.
  - PV even-slot streams in the kt loop; odd-slot P tiles are retained in
    SBUF and PV-odd batches after even's normalize frees the ot banks.
  - Denominator via ones-column in vext (row 64 of O^T); reciprocal on DVE,
    partition-broadcast via small DRAM bounce, normalize on DVE.
  - PSUM->SBUF copies on GpSimd; proj results converted to fp16 on DVE.
  - ALiBi decay (reference adds slope*(i-j), so old keys dominate): keys with
    slope_h*k > ~45 dropped; per-slot key-tile caps SNKT = [2, 6, 16, 16].
"""

import math

import numpy as np

E = 1024
H = 16
DH = 64
T = 2048
BATCH = 2
LORA_S = 2.0
SNKT = [2, 6, 16, 16]   # per-slot key-tile caps (uniform across cores)
MQ = 2                  # merged q-chunks of 1024

_NC_CACHE = {}


def _slopes():
    start = 2 ** (-2 ** (-(math.log2(H) - 3)))
    return np.array([start * start**i for i in range(H)], dtype=np.float64)


def _smin(tt):
    """Lowest slot that still needs key-tile tt."""
    for s in range(4):
        if tt < SNKT[s]:
            return s
    return 4


def _build_nc():
    if "nc" in _NC_CACHE:
        return _NC_CACHE["nc"]

    from concourse.bacc import Bacc
    import concourse.tile as tile
    from concourse import mybir

    f16 = mybir.dt.float16
    f32 = mybir.dt.float32
    EXP = mybir.ActivationFunctionType.Exp

    nc = Bacc()

    xT_d = nc.dram_tensor("xT", [E, T], f16, kind="ExternalInput")
    wqkv_d = nc.dram_tensor("wqkvT", [E, 768], f16, kind="ExternalInput")
    wp_d = nc.dram_tensor("wpT", [256, E], f16, kind="ExternalInput")
    bias_d = nc.dram_tensor("expbias", [128, 64], f32, kind="ExternalInput")
    mask_d = nc.dram_tensor("masktri", [128, 128], f16, kind="ExternalInput")
    out_d = nc.dram_tensor("outp", [T, E], f16, kind="ExternalOutput")
    rb_d = nc.dram_tensor("rbounce", [8, 1024], f32, kind="Internal")

    with tile.TileContext(nc) as tc:
        with (
            tc.tile_pool(name="persist", bufs=1) as pp,
            tc.tile_pool(name="pe_pool", bufs=3) as pep,
            tc.tile_pool(name="po_pool", bufs=16) as pop,
            tc.tile_pool(name="rp", bufs=2) as rp,
            tc.tile_pool(name="osp", bufs=4) as osp,
            tc.tile_pool(name="stE", bufs=1, space="PSUM") as stEp,
            tc.tile_pool(name="stO", bufs=1, space="PSUM") as stOp,
            tc.tile_pool(name="otp", bufs=1, space="PSUM") as otp,
            tc.tile_pool(name="auxp", bufs=2, space="PSUM") as auxp,
        ):
            # ---- persistent SBUF ----
            xsb = pp.tile([128, 8, T], f16, name="xsb")
            wsb = pp.tile([128, 8, 768], f16, name="wsb")
            wpsb = pp.tile([128, 2, E], f16, name="wpsb")
            bias_sb = pp.tile([128, 64], f32, name="bias")
            mask_sb = pp.tile([128, 128], f16, name="masktri")
            qT = [[pp.tile([128, 512], f16, name=f"qT{p}_{c}") for c in range(4)]
                  for p in range(2)]
            kT = [[pp.tile([128, 512], f16, name=f"kT{p}_{c}")
                   if (p == 1 or c < 2) else None for c in range(4)]
                  for p in range(2)]
            vext = [pp.tile([128, 4, 65], f16, name=f"vext{t}") for t in range(16)]
            on_t = [[pp.tile([128, 1024], f16, name=f"on{p}_{m}") for m in range(MQ)]
                    for p in range(2)]

            # dummy exp to pull the ACT table load off the critical path
            scr = pp.tile([1, 2], f32, name="scr")
            nc.vector.memset(scr, 0.0)
            nc.scalar.activation(out=scr[0:1, 1:2], in_=scr[0:1, 0:1], func=EXP)

            for t in range(16):
                nc.gpsimd.memset(vext[t], 1.0)  # ones col preset; v overwrites rest

            # ---- input DMAs: spread across queues for parallel transfer ----
            nc.gpsimd.dma_start(out=bias_sb, in_=bias_d[:, :])
            nc.gpsimd.dma_start(out=mask_sb, in_=mask_d[:, :])
            x3 = xT_d.rearrange("(k p) t -> p k t", p=128)
            w3 = wqkv_d.rearrange("(k p) c -> p k c", p=128)
            wp3 = wp_d.rearrange("(k p) c -> p k c", p=128)
            nc.sync.dma_start(out=wsb, in_=w3)
            nc.scalar.dma_start(out=xsb[:, :, 0:512], in_=x3[:, :, 0:512])
            nc.sync.dma_start(out=xsb[:, :, 512:1024], in_=x3[:, :, 512:1024])
            nc.scalar.dma_start(out=xsb[:, :, 1024:1536], in_=x3[:, :, 1024:1536])
            nc.sync.dma_start(out=xsb[:, :, 1536:2048], in_=x3[:, :, 1536:2048])
            nc.gpsimd.dma_start(out=wpsb, in_=wp3)

            def phase1(mq):
                """QKV projections for q-chunks 2mq, 2mq+1 (v key-tiles 8mq..8mq+7)."""
                with nc.named_scope(f"qkv_mq{mq}"):
                    for ncu in (2 * mq, 2 * mq + 1):
                        for wofs, dst in ((0, qT), (256, kT)):
                            for mt in range(2):
                                if dst[mt][ncu] is None:
                                    continue
                                nw = 256 if (wofs == 256 and mt == 0 and ncu == 1) else 512
                                acc = auxp.tile([128, 512], f32, tag="aux",
                                                name=f"qk{wofs}_{mt}_{ncu}")
                                for kt in range(8):
                                    nc.tensor.matmul(
                                        acc[:, 0:nw],
                                        wsb[:, kt, wofs + mt * 128:wofs + (mt + 1) * 128],
                                        xsb[:, kt, ncu * 512:ncu * 512 + nw],
                                        start=(kt == 0), stop=(kt == 7),
                                    )
                                nc.vector.tensor_copy(out=dst[mt][ncu][:, 0:nw],
                                                      in_=acc[:, 0:nw])
                        for tt in range(4 * ncu, 4 * ncu + 4):
                            s0 = _smin(tt)
                            nw = (4 - s0) * 64
                            acc = auxp.tile([128, 512], f32, tag="aux", name=f"vacc{tt}")
                            for kt in range(8):
                                nc.tensor.matmul(
                                    acc[:, 0:nw],
                                    xsb[:, kt, tt * 128:(tt + 1) * 128],
                                    wsb[:, kt, 512 + s0 * 64:768],
                                    start=(kt == 0), stop=(kt == 7),
                                )
                            nc.vector.tensor_copy(
                                out=vext[tt][:, s0:4, 0:64],
                                in_=acc[:, 0:nw].rearrange("p (s d) -> p s d", d=64))

            def norm(ot, mq, pair, s, r0):
                """on[r0:r0+64] = ot[0:64] / ot[64] (denominator row)."""
                row = 4 * mq + s
                sum_sb = rp.tile([1, 1024], f32, tag="sum", name=f"sum_{mq}_{s}")
                nc.vector.tensor_copy(out=sum_sb, in_=ot[64:65, 0:1024])
                rec = rp.tile([1, 1024], f32, tag="rec", name=f"rec_{mq}_{s}")
                nc.vector.reciprocal_approx_fast(out=rec, in_=sum_sb)
                nc.sync.dma_start(out=rb_d[row:row + 1, :], in_=rec)
                bcs = rp.tile([128, 1024], f32, tag="bcs", name=f"bcs_{mq}_{s}")
                nc.sync.dma_start(
                    out=bcs[r0:r0 + 64, :],
                    in_=rb_d[row:row + 1, :].to_broadcast([64, 1024]))
                nc.vector.tensor_mul(
                    out=on_t[pair][mq][r0:r0 + 64, :],
                    in0=ot[0:64, :], in1=bcs[r0:r0 + 64, :])

            def attention(mq):
                for pair in (1, 0):  # big-exp pair first
                    se, so = 2 * pair, 2 * pair + 1
                    pt = pair
                    nkt_e = min(SNKT[se], 8 * mq + 8)
                    nkt_o = min(SNKT[so], 8 * mq + 8)
                    ot_e = otp.tile([128, 1024], f32, tag="ot", name=f"ote_{mq}_{pair}")
                    p_o_tiles = []
                    with nc.named_scope(f"attn_mq{mq}_p{pair}"):
                        for kt in range(nkt_o):
                            j0 = max(0, (kt - 8 * mq) * 128)
                            kc, ko = kt // 4, (kt % 4) * 128
                            st_e = (stEp.tile([128, 1024], f32, tag="st",
                                              name=f"ste_{mq}_{pair}_{kt}")
                                    if kt < nkt_e else None)
                            st_o = stOp.tile([128, 1024], f32, tag="st",
                                             name=f"sto_{mq}_{pair}_{kt}")
                            for h in (0, 1):
                                lo, hi = max(j0, 512 * h), 512 * (h + 1)
                                if lo >= hi:
                                    continue
                                for st, r0 in ((st_e, 0), (st_o, 64)):
                                    if st is None:
                                        continue
                                    nc.tensor.matmul(
                                        st[:, lo:hi],
                                        kT[pt][kc][r0:r0 + 64, ko:ko + 128],
                                        qT[pt][2 * mq + h][r0:r0 + 64, lo - 512 * h:512],
                                        start=True, stop=True,
                                    )
                            for st, s, r0, is_e in ((st_e, se, 0, True),
                                                    (st_o, so, 64, False)):
                                if st is None:
                                    continue
                                pool, tg = (pep, "pe") if is_e else (pop, "po")
                                p_t = pool.tile([128, 1024], f16, tag=tg,
                                                name=f"p{tg}_{mq}_{pair}_{kt}")
                                nc.scalar.activation(
                                    out=p_t[:, j0:1024], in_=st[:, j0:1024],
                                    func=EXP,
                                    bias=bias_sb[:, s * 16 + kt:s * 16 + kt + 1],
                                    scale=0.125,
                                )
                                if kt >= 8 * mq:
                                    nc.gpsimd.tensor_mul(
                                        out=p_t[:, j0:j0 + 128],
                                        in0=p_t[:, j0:j0 + 128], in1=mask_sb)
                                if is_e:
                                    for h in (0, 1):
                                        lo, hi = max(j0, 512 * h), 512 * (h + 1)
                                        if lo >= hi:
                                            continue
                                        last = kt == ((min(nkt_e, 8 * mq + 4) - 1)
                                                      if h == 0 else nkt_e - 1)
                                        nc.tensor.matmul(
                                            ot_e[0:65, lo:hi],
                                            vext[kt][:, se, :], p_t[:, lo:hi],
                                            start=(kt == 0), stop=last,
                                        )
                                else:
                                    p_o_tiles.append((kt, j0, p_t))
                        norm(ot_e, mq, pair, se, 0)
                        ot_o = otp.tile([128, 1024], f32, tag="ot",
                                        name=f"oto_{mq}_{pair}")
                        for kt, j0, p_t in p_o_tiles:
                            for h in (0, 1):
                                lo, hi = max(j0, 512 * h), 512 * (h + 1)
                                if lo >= hi:
                                    continue
                                last = kt == ((min(nkt_o, 8 * mq + 4) - 1)
                                              if h == 0 else nkt_o - 1)
                                nc.tensor.matmul(
                                    ot_o[0:65, lo:hi],
                                    vext[kt][:, so, :], p_t[:, lo:hi],
                                    start=(kt == 0), stop=last,
                                )
                        norm(ot_o, mq, pair, so, 64)

            def proj(mq):
                with nc.named_scope(f"proj_mq{mq}"):
                    for tloc in range(8):
                        tt = 8 * mq + tloc
                        for ech in range(2):
                            pacc = auxp.tile([128, 512], f32, tag="aux",
                                             name=f"pacc_{tt}_{ech}")
                            for pt in range(2):
                                nc.tensor.matmul(
                                    pacc,
                                    on_t[pt][mq][:, tloc * 128:(tloc + 1) * 128],
                                    wpsb[:, pt, ech * 512:(ech + 1) * 512],
                                    start=(pt == 0), stop=(pt == 1),
                                )
                            osb = osp.tile([128, 512], f16, tag="osb",
                                           name=f"osb_{tt}_{ech}")
                            nc.vector.tensor_copy(out=osb, in_=pacc)
                            nc.sync.dma_start(
                                out=out_d[tt * 128:(tt + 1) * 128,
                                          ech * 512:(ech + 1) * 512],
                                in_=osb)

            phase1(0)
            attention(0)
            phase1(1)
            proj(0)
            attention(1)
            proj(1)

    nc.finalize()
    _NC_CACHE["nc"] = nc
    return nc


def _prep_core_inputs(x, Wq, Aq, Bq, Wk, Ak, Bk, Wv, Av, Bv, Wp):
    """Host-side prep: LoRA fold, transposes, per-core slices."""
    slopes = _slopes()
    wq_m = Wq.astype(np.float64) + LORA_S * (Aq.astype(np.float64) @ Bq.astype(np.float64))
    wk_m = Wk.astype(np.float64) + LORA_S * (Ak.astype(np.float64) @ Bk.astype(np.float64))
    wv_m = Wv.astype(np.float64) + LORA_S * (Av.astype(np.float64) @ Bv.astype(np.float64))

    # triangle mask[p, j] = 1 if p <= j (valid, key on/before query) else 0
    p_i = np.arange(128)[:, None]
    j_i = np.arange(128)[None, :]
    masktri = np.ascontiguousarray((p_i <= j_i).astype(np.float16))

    in_maps = []
    for c in range(8):
        b, g = divmod(c, 4)
        heads = [g, 4 + g, 8 + g, 12 + g]
        rows = np.concatenate([np.arange(h * DH, (h + 1) * DH) for h in heads])
        xT = np.ascontiguousarray(x[b].T.astype(np.float16))
        wqkvT = np.ascontiguousarray(np.concatenate(
            [wq_m[rows, :].T, wk_m[rows, :].T, wv_m[rows, :].T],
            axis=1).astype(np.float16))
        wpT = np.ascontiguousarray(Wp[:, rows].T.astype(np.float16))
        bias = np.zeros((128, 64), dtype=np.float32)
        for s, h in enumerate(heads):
            for kt in range(16):
                bias[:, s * 16 + kt] = -slopes[h] * (kt * 128 + np.arange(128))
        in_maps.append({
            "xT": xT, "wqkvT": wqkvT, "wpT": wpT,
            "expbias": bias, "masktri": masktri,
        })
    return in_maps


def _run(in_maps, trace=False, **kw):
    from concourse.bass_utils import run_bass_kernel_spmd
    nc = _build_nc()
    return run_bass_kernel_spmd(nc, in_maps, core_ids=list(range(8)), trace=trace, **kw)


def kernel(x, Wq, Aq, Bq, Wk, Ak, Bk, Wv, Av, Bv, Wp):
    in_maps = _prep_core_inputs(x, Wq, Aq, Bq, Wk, Ak, Bk, Wv, Av, Bv, Wp)
    res = _run(in_maps)
    out = np.zeros((BATCH, T, E), dtype=np.float32)
    for c in range(8):
        out[c // 4] += res.results[c]["outp"].astype(np.float32)
    return out
